# revision 5
# baseline (speedup 1.0000x reference)
"""DLSPooling Trainium2 kernel.

Math (reference drops out_adj, so the dense pooled adjacency is never needed):
    m   = elu(x @ W_msg)                          [N, K]
    agg = segment_sum(m[src], dst)                [N, K]
    s   = tanh(agg + x @ W_root + b)              [N, K]
    S   = softmax(s, axis=-1)
    out[g] = S_g^T X_g                            [K, C] per graph
    returns (out.reshape(B*K, C), edge_index_out, batch_out)

Sharding: 8 graphs per NeuronCore (data-parallel over B=64 graphs).
Edges stay within graphs (PyG batched-graph invariant), so per-graph
aggregation is agg_g = A_g^T m_g with A_g the dense [1024,1024] adjacency
count matrix, which each device owns (built host-side from the integer edge
list; counts are small integers, exact in fp16).  The rare/impossible case of
cross-graph edges is handled exactly via an additive correction input.

Precision: messages are split hi/lo into two fp16 halves so the fp16 tensor
engine matmuls recover ~fp32 accuracy; x is shipped as fp16 hi/lo pairs for
the pooling matmul.  tanh output is in (-1,1) so softmax needs no max
subtraction.
"""

import sys

import numpy as np

sys.path.insert(0, "/opt/trn_rl_repo")

import concourse.bacc as bacc
import concourse.mybir as mybir
import concourse.tile as tile
from concourse import bass_utils

B = 64          # graphs
N_PER = 1024    # nodes per graph
C = 128         # channels
K = 64          # clusters
N = B * N_PER
NCORES = 8
GPC = B // NCORES          # graphs per core = 8
NODES = GPC * N_PER        # nodes per core = 8192
NT = NODES // 128          # 128-node tiles per core = 64
SC = N_PER // 128          # node chunks per graph = 8
MS = 130                   # mhl per-tile stride: 64 hi + 64 lo + 1 neg-ones + 1 pad

f32 = mybir.dt.float32
f16 = mybir.dt.float16
AF = mybir.ActivationFunctionType
ALU = mybir.AluOpType

_CACHE = {}
TRACE = False
TRACE_DIR = None
LAST_EXEC_NS = None


def _build(use_extra: bool):
    nc = bacc.Bacc("TRN2", target_bir_lowering=False, debug=False,
                   num_devices=NCORES)
    xT_d = nc.dram_tensor("xT", [128, NODES], f32, kind="ExternalInput").ap()
    xhl_d = nc.dram_tensor("xhl", [128, NT * 256], f16, kind="ExternalInput").ap()
    A_d = nc.dram_tensor("A", [GPC, 128, SC * N_PER], f16, kind="ExternalInput").ap()
    wcat_d = nc.dram_tensor("Wcat", [128, 128], f32, kind="ExternalInput").ap()
    b4_d = nc.dram_tensor("b4", [1, 256], f32, kind="ExternalInput").ap()
    if use_extra:
        extra_d = nc.dram_tensor("extra", [128, NT * K], f32, kind="ExternalInput").ap()
    out_d = nc.dram_tensor("out", [GPC * K, C], f32, kind="ExternalOutput").ap()

    with tile.TileContext(nc) as tc:
        with (
            tc.tile_pool(name="const", bufs=1) as cpool,
            tc.tile_pool(name="big", bufs=1) as big,
            tc.tile_pool(name="apool", bufs=2) as apool,
            tc.tile_pool(name="mtmp", bufs=2) as mtmp,
            tc.tile_pool(name="stmp", bufs=3) as stmp,
        ):
            wcat = cpool.tile([128, 128], f32)
            b4row = cpool.tile([1, 256], f32)
            ones1 = cpool.tile([1, 128], f32)
            b_bc4 = cpool.tile([128, 256], f32)

            xT = big.tile([128, NODES], f32)
            xhl = big.tile([128, NT * 256], f16)
            mhl = big.tile([128, NT * MS], f16)
            xwrb = big.tile([128, NT * K], f32)
            s_sb = big.tile([128, NT * K], f32)
            e_sb = big.tile([128, NT * K], f16)
            ep_sb = big.tile([128, NT * K], f16)
            rs = big.tile([128, NT], f32)
            rr = big.tile([128, NT], f32)
            outsb = big.tile([64, GPC * C], f32)
            if use_extra:
                extra_sb = big.tile([128, NT * K], f32)

            nc.sync.dma_start(wcat[:], wcat_d[:])
            nc.sync.dma_start(b4row[:], b4_d[:])
            for i in range(8):
                nc.sync.dma_start(xT[:, i * 1024:(i + 1) * 1024],
                                  xT_d[:, i * 1024:(i + 1) * 1024])
            if use_extra:
                nc.sync.dma_start(extra_sb[:], extra_d[:])
            nc.vector.memset(ones1[:], 1.0)
            # -1 columns (and pad) of the message buffer, one strided memset
            mhl3 = mhl[:].rearrange("p (t c) -> p t c", c=MS)
            nc.gpsimd.memset(mhl3[:, :, 128:130], -1.0)

            # b broadcast to all partitions, tiled 4x along free dim
            with tc.tile_pool(name="psum_b", bufs=1, space="PSUM") as psb, \
                 tc.tile_pool(name="psum_m", bufs=3, space="PSUM") as psm:
                pb = psb.tile([128, 256], f32)
                nc.tensor.matmul(pb[:], ones1[:], b4row[:], start=True, stop=True)
                nc.scalar.activation(b_bc4[:], pb[:], AF.Copy)
                b_bc43 = b_bc4[:].rearrange("p (t k) -> p t k", k=K)

                # ---- phase M: m_pre = x @ [W_msg | W_root], elu, hi/lo split
                xwrb3 = xwrb[:].rearrange("p (t k) -> p t k", k=K)
                for bi in range(NT // 4):
                    pm = psm.tile([128, 512], f32)
                    for j in range(4):
                        t = bi * 4 + j
                        nc.tensor.matmul(pm[:, j * 128:(j + 1) * 128],
                                         xT[:, t * 128:(t + 1) * 128],
                                         wcat[:], start=True, stop=True)
                    pm3 = pm[:].rearrange("p (t c) -> p t c", c=128)
                    mv = mhl3[:, bi * 4:(bi + 1) * 4, :]
                    t_e = mtmp.tile([128, 256], f32, tag="t_e")
                    t_e3 = t_e[:].rearrange("p (t k) -> p t k", k=K)
                    t_r = mtmp.tile([128, 256], f32, tag="t_r")
                    t_r3 = t_r[:].rearrange("p (t k) -> p t k", k=K)
                    t_q = mtmp.tile([128, 256], f32, tag="t_q")
                    t_q3 = t_q[:].rearrange("p (t k) -> p t k", k=K)
                    t_h = mtmp.tile([128, 256], f32, tag="t_h")
                    t_h3 = t_h[:].rearrange("p (t k) -> p t k", k=K)
                    # q = elu(m) + 1 = relu(m) + min(exp(m), 1)
                    nc.scalar.activation(t_e3[:], pm3[:, :, 0:64], AF.Exp)
                    nc.vector.tensor_scalar_max(t_r3[:], pm3[:, :, 0:64], 0.0)
                    nc.vector.scalar_tensor_tensor(t_q3[:], t_e3[:], 1.0, t_r3[:],
                                                   op0=ALU.min, op1=ALU.add)
                    nc.vector.tensor_copy(mv[:, :, 0:64], t_q3[:])
                    nc.scalar.activation(t_h3[:], mv[:, :, 0:64], AF.Copy)
                    nc.vector.tensor_tensor(mv[:, :, 64:128], t_q3[:],
                                            t_h3[:], op=ALU.subtract)
                    nc.vector.tensor_tensor(xwrb3[:, bi * 4:(bi + 1) * 4, :],
                                            pm3[:, :, 64:128], b_bc43[:],
                                            op=ALU.add)

            if use_extra:
                extra3 = extra_sb[:].rearrange("p (t k) -> p t k", k=K)

            with tc.tile_pool(name="psum_a", bufs=2, space="PSUM") as psa, \
                 tc.tile_pool(name="psum_p", bufs=2, space="PSUM") as psp:
                # ---- phase A: agg = A^T q (hi|lo|[-deg]) per graph, + combine
                for g in range(GPC):
                    a_sb = apool.tile([128, SC * N_PER], f16)
                    nc.sync.dma_start(a_sb[:], A_d[g])
                    for h in range(2):
                        pa = psa.tile([128, 1024], f32)
                        for dc4 in range(4):
                            dc = h * 4 + dc4
                            for sc in range(SC):
                                t = g * SC + sc
                                nc.tensor.matmul(
                                    pa[:, dc4 * 256:dc4 * 256 + 129],
                                    a_sb[:, sc * N_PER + dc * 128:
                                         sc * N_PER + dc * 128 + 128],
                                    mhl[:, t * MS:t * MS + 129],
                                    start=(sc == 0), stop=(sc == SC - 1))
                        for dc4 in range(4):
                            t = g * SC + h * 4 + dc4
                            tt = stmp.tile([128, 64], f32, tag="cmb")
                            # (agg_hi + (-wdeg)) + (xWr + b)
                            nc.vector.scalar_tensor_tensor(
                                tt[:], pa[:, dc4 * 256:dc4 * 256 + 64],
                                pa[:, dc4 * 256 + 128:dc4 * 256 + 129],
                                xwrb[:, t * K:(t + 1) * K],
                                op0=ALU.add, op1=ALU.add)
                            if use_extra:
                                nc.vector.tensor_tensor(
                                    tt[:], tt[:], extra3[:, t, :], op=ALU.add)
                            nc.vector.tensor_tensor(
                                s_sb[:, t * K:(t + 1) * K], tt[:],
                                pa[:, dc4 * 256 + 64:dc4 * 256 + 128],
                                op=ALU.add)

                # ---- phase S: tanh, exp, row sums, normalize
                nc.scalar.activation(s_sb[:], s_sb[:], AF.Tanh)
                nc.scalar.activation(e_sb[:], s_sb[:], AF.Exp)
                e3 = e_sb[:].rearrange("p (t k) -> p t k", k=K)
                nc.vector.tensor_reduce(rs[:], e3[:], axis=mybir.AxisListType.X,
                                        op=ALU.add)
                nc.vector.reciprocal(rr[:], rs[:])
                for t in range(NT):
                    nc.vector.tensor_scalar_mul(ep_sb[:, t * K:(t + 1) * K],
                                                e_sb[:, t * K:(t + 1) * K],
                                                rr[:, t:t + 1])

                # xhl arrives late; schedule its DMA after the A matrices
                for i in range(8):
                    nc.sync.dma_start(xhl[:, i * 2048:(i + 1) * 2048],
                                      xhl_d[:, i * 2048:(i + 1) * 2048])

                # ---- phase P: out_g = S^T x_hi + S^T x_lo (one PSUM group)
                for g in range(GPC):
                    pp = psp.tile([64, 128], f32)
                    for half in range(2):
                        for sc in range(SC):
                            t = g * SC + sc
                            nc.tensor.matmul(
                                pp[:], ep_sb[:, t * K:(t + 1) * K],
                                xhl[:, t * 256 + half * 128:
                                    t * 256 + half * 128 + 128],
                                start=(half == 0 and sc == 0),
                                stop=(half == 1 and sc == SC - 1))
                    nc.scalar.activation(outsb[:, g * C:(g + 1) * C], pp[:],
                                         AF.Copy)

                out_v = out_d.rearrange("(g k) c -> k g c", g=GPC)
                osb_v = outsb[:].rearrange("k (g c) -> k g c", g=GPC)
                nc.sync.dma_start(out_v, osb_v)

    nc.compile()
    return nc


def _get_nc(use_extra: bool):
    if use_extra not in _CACHE:
        _CACHE[use_extra] = _build(use_extra)
    return _CACHE[use_extra]


def _elu(v):
    return np.where(v > 0, v, np.expm1(np.minimum(v, 0.0)))


def kernel(x, edge_index, batch, W_msg, W_root, b):
    x = np.ascontiguousarray(np.asarray(x, dtype=np.float32))
    W_msg = np.asarray(W_msg, dtype=np.float32)
    W_root = np.asarray(W_root, dtype=np.float32)
    b = np.asarray(b, dtype=np.float32)
    ei = np.asarray(edge_index)
    src = ei[0].astype(np.int64, copy=False)
    dst = ei[1].astype(np.int64, copy=False)

    cross = (src >> 10) != (dst >> 10)
    n_cross = int(cross.sum())
    if n_cross:
        sw, dw = src[~cross], dst[~cross]
    else:
        sw, dw = src, dst
    key = (sw << 10) + (dw & 1023)
    counts = np.bincount(key, minlength=N * N_PER)
    cmax = counts.max(initial=0)
    assert cmax <= 2047, f"adjacency count {cmax} not exact in fp16"
    # A[g, sc, p, d] -> [g, p, sc*1024 + d]
    A16 = (counts.astype(np.float16)
           .reshape(B, SC, 128, N_PER).transpose(0, 2, 1, 3)
           .reshape(B, 128, SC * N_PER))

    use_extra = n_cross > 0
    if use_extra:
        agg_extra = np.zeros((N, K), np.float32)
        sc_, dc_ = src[cross], dst[cross]
        m_rows = _elu(x[sc_] @ W_msg).astype(np.float32)
        np.add.at(agg_extra, dc_, m_rows)
        # [n, k] -> per core [128, t, k] -> [128, t*k]
        extra_pc = (agg_extra.reshape(NCORES, NT, 128, K).transpose(0, 2, 1, 3)
                    .reshape(NCORES, 128, NT * K))

    wcat = np.ascontiguousarray(np.concatenate([W_msg, W_root], axis=1),
                                dtype=np.float32)
    b4 = np.tile(b, 4)[None, :].astype(np.float32)

    xr = x.reshape(NCORES, NT, 128, C)
    xhi = xr.astype(np.float16)
    xlo = (xr - xhi.astype(np.float32)).astype(np.float16)
    # [core, t, p, 256] -> [core, p, t*256]
    xhl = (np.concatenate([xhi, xlo], axis=3).transpose(0, 2, 1, 3)
           .reshape(NCORES, 128, NT * 256))
    xT = np.ascontiguousarray(
        x.reshape(NCORES, NODES, C).transpose(0, 2, 1))

    in_maps = []
    for c in range(NCORES):
        m = {
            "xT": np.ascontiguousarray(xT[c]),
            "xhl": np.ascontiguousarray(xhl[c]),
            "A": np.ascontiguousarray(A16[c * GPC:(c + 1) * GPC]),
            "Wcat": wcat,
            "b4": b4,
        }
        if use_extra:
            m["extra"] = np.ascontiguousarray(extra_pc[c])
        in_maps.append(m)

    nc = _get_nc(use_extra)
    res = bass_utils.run_bass_kernel_spmd(nc, in_maps, list(range(NCORES)),
                                          trace=TRACE, tmpdir=TRACE_DIR)
    global LAST_EXEC_NS
    LAST_EXEC_NS = res.exec_time_ns
    x_out = np.concatenate([res.results[c]["out"] for c in range(NCORES)],
                           axis=0)

    grid = np.stack(np.meshgrid(np.arange(K), np.arange(K), indexing="ij"),
                    0).reshape(2, -1)
    offs = (np.arange(B) * K)[None, :, None]
    edge_index_out = (grid[:, None, :] + offs).reshape(2, -1).astype(np.int32)
    batch_out = np.repeat(np.arange(B), K).astype(np.int32)
    return x_out.astype(np.float32), edge_index_out, batch_out


# revision 6
# speedup vs baseline: 1.2065x; 1.2065x over previous
"""DLSPooling Trainium2 kernel.

Math (reference drops out_adj, so the dense pooled adjacency is never needed):
    m   = elu(x @ W_msg)                          [N, K]
    agg = segment_sum(m[src], dst)                [N, K]
    s   = tanh(agg + x @ W_root + b)              [N, K]
    S   = softmax(s, axis=-1)
    out[g] = S_g^T X_g                            [K, C] per graph
    returns (out.reshape(B*K, C), edge_index_out, batch_out)

Sharding: 8 graphs per NeuronCore (data-parallel over B=64 graphs).
Edges stay within graphs (PyG batched-graph invariant), so per-graph
aggregation is agg_g = A_g^T m_g with A_g the dense [1024,1024] adjacency
count matrix, which each device owns (built host-side from the integer edge
list; counts are small integers, exact in fp16).  The rare/impossible case of
cross-graph edges is handled exactly via an additive correction input.

Precision: messages are split hi/lo into two fp16 halves so the fp16 tensor
engine matmuls recover ~fp32 accuracy; x is shipped as fp16 hi/lo pairs for
the pooling matmul.  tanh output is in (-1,1) so softmax needs no max
subtraction.
"""

import sys

import numpy as np

sys.path.insert(0, "/opt/trn_rl_repo")

import concourse.bacc as bacc
import concourse.mybir as mybir
import concourse.tile as tile
from concourse import bass_utils

B = 64          # graphs
N_PER = 1024    # nodes per graph
C = 128         # channels
K = 64          # clusters
N = B * N_PER
NCORES = 8
GPC = B // NCORES          # graphs per core = 8
NODES = GPC * N_PER        # nodes per core = 8192
NT = NODES // 128          # 128-node tiles per core = 64
SC = N_PER // 128          # node chunks per graph = 8
MS = 130                   # mhl per-tile stride: 64 hi + 64 lo + 1 neg-ones + 1 pad

f32 = mybir.dt.float32
f16 = mybir.dt.float16
f8 = mybir.dt.float8e4
AF = mybir.ActivationFunctionType
ALU = mybir.AluOpType

_CACHE = {}
TRACE = False
TRACE_DIR = None
LAST_EXEC_NS = None


def _build(use_extra: bool, a_f8: bool = True):
    nc = bacc.Bacc("TRN2", target_bir_lowering=False, debug=False,
                   num_devices=NCORES)
    xT_d = nc.dram_tensor("xT", [128, NODES], f32, kind="ExternalInput").ap()
    xhl_d = nc.dram_tensor("xhl", [128, NT * 256], f16, kind="ExternalInput").ap()
    a_dt = f8 if a_f8 else f16
    A_d = nc.dram_tensor("A", [GPC, 128, SC * N_PER], a_dt, kind="ExternalInput").ap()
    wcat_d = nc.dram_tensor("Wcat", [128, 128], f32, kind="ExternalInput").ap()
    b4_d = nc.dram_tensor("b4", [1, 256], f32, kind="ExternalInput").ap()
    if use_extra:
        extra_d = nc.dram_tensor("extra", [128, NT * K], f32, kind="ExternalInput").ap()
    out_d = nc.dram_tensor("out", [GPC * K, C], f32, kind="ExternalOutput").ap()

    with tile.TileContext(nc) as tc:
        with (
            tc.tile_pool(name="const", bufs=1) as cpool,
            tc.tile_pool(name="big", bufs=1) as big,
            tc.tile_pool(name="apool", bufs=3) as apool,
            tc.tile_pool(name="mtmp", bufs=2) as mtmp,
            tc.tile_pool(name="stmp", bufs=3) as stmp,
        ):
            wcat = cpool.tile([128, 128], f32)
            b4row = cpool.tile([1, 256], f32)
            ones1 = cpool.tile([1, 128], f32)
            b_bc4 = cpool.tile([128, 256], f32)

            xT = big.tile([128, NODES], f32)
            xhl = big.tile([128, NT * 256], f16)
            mhl = big.tile([128, NT * MS], f16)
            xwrb = big.tile([128, NT * K], f32)
            s_sb = big.tile([128, NT * K], f32)
            e_sb = big.tile([128, NT * K], f16)
            ep_sb = big.tile([128, NT * K], f16)
            rs = big.tile([128, NT], f32)
            rr = big.tile([128, NT], f32)
            outsb = big.tile([64, GPC * C], f32)
            if use_extra:
                extra_sb = big.tile([128, NT * K], f32)

            nc.sync.dma_start(wcat[:], wcat_d[:])
            nc.sync.dma_start(b4row[:], b4_d[:])
            for i in range(8):
                nc.sync.dma_start(xT[:, i * 1024:(i + 1) * 1024],
                                  xT_d[:, i * 1024:(i + 1) * 1024])
            if use_extra:
                nc.sync.dma_start(extra_sb[:], extra_d[:])
            nc.vector.memset(ones1[:], 1.0)
            # -1 columns (and pad) of the message buffer, one strided memset
            mhl3 = mhl[:].rearrange("p (t c) -> p t c", c=MS)
            nc.gpsimd.memset(mhl3[:, :, 128:130], -1.0)

            # b broadcast to all partitions, tiled 4x along free dim
            with tc.tile_pool(name="psum_b", bufs=1, space="PSUM") as psb, \
                 tc.tile_pool(name="psum_m", bufs=3, space="PSUM") as psm:
                pb = psb.tile([128, 256], f32)
                nc.tensor.matmul(pb[:], ones1[:], b4row[:], start=True, stop=True)
                nc.scalar.activation(b_bc4[:], pb[:], AF.Copy)
                b_bc43 = b_bc4[:].rearrange("p (t k) -> p t k", k=K)

                # ---- phase M: m_pre = x @ [W_msg | W_root], elu, hi/lo split
                xwrb3 = xwrb[:].rearrange("p (t k) -> p t k", k=K)
                for bi in range(NT // 4):
                    pm = psm.tile([128, 512], f32)
                    for j in range(4):
                        t = bi * 4 + j
                        nc.tensor.matmul(pm[:, j * 128:(j + 1) * 128],
                                         xT[:, t * 128:(t + 1) * 128],
                                         wcat[:], start=True, stop=True)
                    pm3 = pm[:].rearrange("p (t c) -> p t c", c=128)
                    mv = mhl3[:, bi * 4:(bi + 1) * 4, :]
                    t_e = mtmp.tile([128, 256], f32, tag="t_e")
                    t_e3 = t_e[:].rearrange("p (t k) -> p t k", k=K)
                    t_r = mtmp.tile([128, 256], f32, tag="t_r")
                    t_r3 = t_r[:].rearrange("p (t k) -> p t k", k=K)
                    t_q = mtmp.tile([128, 256], f32, tag="t_q")
                    t_q3 = t_q[:].rearrange("p (t k) -> p t k", k=K)
                    t_h = mtmp.tile([128, 256], f32, tag="t_h")
                    t_h3 = t_h[:].rearrange("p (t k) -> p t k", k=K)
                    # q = elu(m) + 1 = relu(m) + min(exp(m), 1)
                    nc.scalar.activation(t_e3[:], pm3[:, :, 0:64], AF.Exp)
                    nc.vector.tensor_scalar_max(t_r3[:], pm3[:, :, 0:64], 0.0)
                    nc.vector.scalar_tensor_tensor(t_q3[:], t_e3[:], 1.0, t_r3[:],
                                                   op0=ALU.min, op1=ALU.add)
                    nc.vector.tensor_copy(mv[:, :, 0:64], t_q3[:])
                    nc.scalar.activation(t_h3[:], mv[:, :, 0:64], AF.Copy)
                    nc.vector.tensor_tensor(mv[:, :, 64:128], t_q3[:],
                                            t_h3[:], op=ALU.subtract)
                    nc.vector.tensor_tensor(xwrb3[:, bi * 4:(bi + 1) * 4, :],
                                            pm3[:, :, 64:128], b_bc43[:],
                                            op=ALU.add)

            if use_extra:
                extra3 = extra_sb[:].rearrange("p (t k) -> p t k", k=K)

            with tc.tile_pool(name="psum_a", bufs=2, space="PSUM") as psa, \
                 tc.tile_pool(name="psum_p", bufs=2, space="PSUM") as psp:
                # ---- phase A: agg = A^T q (hi|lo|[-deg]) per graph, + combine
                for g in range(GPC):
                    a_sb = apool.tile([128, SC * N_PER], a_dt)
                    nc.sync.dma_start(a_sb[:], A_d[g])
                    for h in range(2):
                        pa = psa.tile([128, 1024], f32)
                        for dc4 in range(4):
                            dc = h * 4 + dc4
                            for sc in range(SC):
                                t = g * SC + sc
                                nc.tensor.matmul(
                                    pa[:, dc4 * 256:dc4 * 256 + 129],
                                    a_sb[:, sc * N_PER + dc * 128:
                                         sc * N_PER + dc * 128 + 128],
                                    mhl[:, t * MS:t * MS + 129],
                                    start=(sc == 0), stop=(sc == SC - 1))
                        for dc4 in range(4):
                            t = g * SC + h * 4 + dc4
                            tt = stmp.tile([128, 64], f32, tag="cmb")
                            # (agg_hi + (-wdeg)) + (xWr + b)
                            nc.vector.scalar_tensor_tensor(
                                tt[:], pa[:, dc4 * 256:dc4 * 256 + 64],
                                pa[:, dc4 * 256 + 128:dc4 * 256 + 129],
                                xwrb[:, t * K:(t + 1) * K],
                                op0=ALU.add, op1=ALU.add)
                            if use_extra:
                                nc.vector.tensor_tensor(
                                    tt[:], tt[:], extra3[:, t, :], op=ALU.add)
                            nc.vector.tensor_tensor(
                                s_sb[:, t * K:(t + 1) * K], tt[:],
                                pa[:, dc4 * 256 + 64:dc4 * 256 + 128],
                                op=ALU.add)
                    nc.scalar.activation(
                        s_sb[:, g * SC * K:(g + 1) * SC * K],
                        s_sb[:, g * SC * K:(g + 1) * SC * K], AF.Tanh)

                # ---- phase S: exp, row sums, normalize
                nc.scalar.activation(e_sb[:], s_sb[:], AF.Exp)
                e3 = e_sb[:].rearrange("p (t k) -> p t k", k=K)
                nc.vector.tensor_reduce(rs[:], e3[:], axis=mybir.AxisListType.X,
                                        op=ALU.add)
                nc.vector.reciprocal(rr[:], rs[:])
                for t in range(NT):
                    nc.vector.tensor_scalar_mul(ep_sb[:, t * K:(t + 1) * K],
                                                e_sb[:, t * K:(t + 1) * K],
                                                rr[:, t:t + 1])

                # xhl arrives late; schedule its DMA after the A matrices
                for i in range(8):
                    nc.sync.dma_start(xhl[:, i * 2048:(i + 1) * 2048],
                                      xhl_d[:, i * 2048:(i + 1) * 2048])

                # ---- phase P: out_g = S^T x_hi + S^T x_lo (one PSUM group)
                for g in range(GPC):
                    pp = psp.tile([64, 128], f32)
                    for half in range(2):
                        for sc in range(SC):
                            t = g * SC + sc
                            nc.tensor.matmul(
                                pp[:], ep_sb[:, t * K:(t + 1) * K],
                                xhl[:, t * 256 + half * 128:
                                    t * 256 + half * 128 + 128],
                                start=(half == 0 and sc == 0),
                                stop=(half == 1 and sc == SC - 1))
                    nc.scalar.activation(outsb[:, g * C:(g + 1) * C], pp[:],
                                         AF.Copy)

                out_v = out_d.rearrange("(g k) c -> k g c", g=GPC)
                osb_v = outsb[:].rearrange("k (g c) -> k g c", g=GPC)
                nc.sync.dma_start(out_v, osb_v)

    nc.compile()
    return nc


def _get_nc(use_extra: bool, a_f8: bool = True):
    key = (use_extra, a_f8)
    if key not in _CACHE:
        _CACHE[key] = _build(use_extra, a_f8)
    return _CACHE[key]


def _elu(v):
    return np.where(v > 0, v, np.expm1(np.minimum(v, 0.0)))


def kernel(x, edge_index, batch, W_msg, W_root, b):
    x = np.ascontiguousarray(np.asarray(x, dtype=np.float32))
    W_msg = np.asarray(W_msg, dtype=np.float32)
    W_root = np.asarray(W_root, dtype=np.float32)
    b = np.asarray(b, dtype=np.float32)
    ei = np.asarray(edge_index)
    src = ei[0].astype(np.int64, copy=False)
    dst = ei[1].astype(np.int64, copy=False)

    cross = (src >> 10) != (dst >> 10)
    n_cross = int(cross.sum())
    if n_cross:
        sw, dw = src[~cross], dst[~cross]
    else:
        sw, dw = src, dst
    key = (sw << 10) + (dw & 1023)
    counts = np.bincount(key, minlength=N * N_PER)
    cmax = counts.max(initial=0)
    assert cmax <= 2047, f"adjacency count {cmax} not exact in fp16"
    a_f8 = cmax <= 16
    import ml_dtypes
    a_np_dt = ml_dtypes.float8_e4m3 if a_f8 else np.float16
    # A[g, sc, p, d] -> [g, p, sc*1024 + d]
    A16 = (counts.astype(a_np_dt)
           .reshape(B, SC, 128, N_PER).transpose(0, 2, 1, 3)
           .reshape(B, 128, SC * N_PER))

    use_extra = n_cross > 0
    if use_extra:
        agg_extra = np.zeros((N, K), np.float32)
        sc_, dc_ = src[cross], dst[cross]
        m_rows = _elu(x[sc_] @ W_msg).astype(np.float32)
        np.add.at(agg_extra, dc_, m_rows)
        # [n, k] -> per core [128, t, k] -> [128, t*k]
        extra_pc = (agg_extra.reshape(NCORES, NT, 128, K).transpose(0, 2, 1, 3)
                    .reshape(NCORES, 128, NT * K))

    wcat = np.ascontiguousarray(np.concatenate([W_msg, W_root], axis=1),
                                dtype=np.float32)
    b4 = np.tile(b, 4)[None, :].astype(np.float32)

    xr = x.reshape(NCORES, NT, 128, C)
    xhi = xr.astype(np.float16)
    xlo = (xr - xhi.astype(np.float32)).astype(np.float16)
    # [core, t, p, 256] -> [core, p, t*256]
    xhl = (np.concatenate([xhi, xlo], axis=3).transpose(0, 2, 1, 3)
           .reshape(NCORES, 128, NT * 256))
    xT = np.ascontiguousarray(
        x.reshape(NCORES, NODES, C).transpose(0, 2, 1))

    in_maps = []
    for c in range(NCORES):
        m = {
            "xT": np.ascontiguousarray(xT[c]),
            "xhl": np.ascontiguousarray(xhl[c]),
            "A": np.ascontiguousarray(A16[c * GPC:(c + 1) * GPC]),
            "Wcat": wcat,
            "b4": b4,
        }
        if use_extra:
            m["extra"] = np.ascontiguousarray(extra_pc[c])
        in_maps.append(m)

    nc = _get_nc(use_extra, a_f8)
    res = bass_utils.run_bass_kernel_spmd(nc, in_maps, list(range(NCORES)),
                                          trace=TRACE, tmpdir=TRACE_DIR)
    global LAST_EXEC_NS
    LAST_EXEC_NS = res.exec_time_ns
    x_out = np.concatenate([res.results[c]["out"] for c in range(NCORES)],
                           axis=0)

    grid = np.stack(np.meshgrid(np.arange(K), np.arange(K), indexing="ij"),
                    0).reshape(2, -1)
    offs = (np.arange(B) * K)[None, :, None]
    edge_index_out = (grid[:, None, :] + offs).reshape(2, -1).astype(np.int32)
    batch_out = np.repeat(np.arange(B), K).astype(np.int32)
    return x_out.astype(np.float32), edge_index_out, batch_out
